# revision 21
# baseline (speedup 1.0000x reference)
"""Class-align loss (segment_reduce) Trainium2 kernel.

Full inputs: f_source [4,256,128,128] f32, f_convert [4,256,128,128] f32,
seg [4,128,128] int32 (values in [0,19)). Output: scalar f32 triplet loss.

Strategy (data-parallel over batch*h-half, 8 shards):
  - Each core processes a [256, 8192] shard of each feature tensor
    (1 batch x 64 h-rows x 128 w).
  - Per 128-pixel group: PE-transpose the two [128c, 128p] blocks into a
    [128p, 256c] SBUF tile; ACT computes per-pixel sum-of-squares
    (Square + accum); per-pixel 1/||x|| is folded into the one-hot class
    weights w[p,k] = (seg[p]==k) * rsqrt(sumsq[p]).
  - PE accumulates S[k,c] += w.T @ xT into a [19,256] PSUM accumulator
    (64 accumulating matmuls per tensor). Same for f_convert.
  - Each core writes its partial [2,19,256] sums; the host sums the 8
    partials and computes the tiny (19-class) normalize + triplet-loss
    epilogue in float64.
"""

import sys

import numpy as np

if "/opt/trn_rl_repo" not in sys.path:
    sys.path.insert(0, "/opt/trn_rl_repo")

import concourse.bass as bass
import concourse.mybir as mybir
import concourse.tile as tile
import concourse.tile_sem_assignment as _tsa

# Cap the SWDGE semaphore round-robin: every sem a kernel touches lands in
# the kernel-tail drain's wait list, and walrus caps sync waits per
# instruction. One lane still pipelines transfers (completion counting only).
_tsa.NUM_SWDGE_GLOBAL_SEMS = 1

# This walrus build encodes at most ONE sync wait per instruction, but
# Tile's kernel-tail drain carries one wait per semaphore the kernel ever
# used. Split the excess waits onto dedicated sequencer NOPs emitted before
# the sem-clearing barrier.
from concourse.vector_clock import ScopedClock


def _split_drain_and_barrier(self, tick_clock, wait_clock):
    nc = self.nc
    drain_inst = nc.sync.drain()
    wait_clock.add_sem_waits(
        drain_inst.ins, ScopedClock({None: tick_clock.global_clock})
    )
    si = drain_inst.ins.sync_info
    if si is not None and len(si.on_wait) > 1:
        waits = list(si.on_wait)
        upds = list(si.on_update)
        drain_inst.ins.sync_info = mybir.SyncInfo(
            on_wait=waits[:1], on_update=upds)
        for k in range(1, len(waits)):
            nop = nc.sync.nop(nofuse=True, hint=f"drain_wait_{k}")
            nop.ins.sync_info = mybir.SyncInfo(
                on_wait=[waits[k]], on_update=[])
    nc.all_engine_barrier()
    assert self.sems is not None
    popped = nc._tile_sem_poison_stack.pop()
    assert popped is self._sem_poison
    nc.clear_and_free_semaphores(list(self.sems.allocated().values()))
    nc.all_engine_barrier()


tile.TileContext._drain_and_barrier = _split_drain_and_barrier
from concourse.bass_utils import run_bass_kernel_spmd
from concourse.tile import add_dep_helper

# Problem constants (hardcoded; kernel.py must be self-contained).
B, C, H, W = 4, 256, 128, 128
N_CLASS = 19
N_CORES = 8
EPS_NORM = 1e-12
EPS_TRIP = 1e-6
MARGIN = 0.2

P = 128                    # SBUF partitions / pixel-group size
NPIX = B * H * W // N_CORES  # 8192 pixels per core
NG = NPIX // P             # 64 pixel groups per core
CHUNK_PIX = 2048           # pixels staged per DMA
NCHUNK = NPIX // CHUNK_PIX # 4
GPC = CHUNK_PIX // P       # 16 groups per chunk

_NC_CACHE = {}


def build_nc():
    f32 = mybir.dt.float32
    i32 = mybir.dt.int32
    nc = bass.Bass(num_swdge_queues=2)

    fs_dram = nc.declare_dram_parameter("f_source", [C, NPIX], f32, isOutput=False)
    aux_dram = nc.declare_dram_parameter("aux", [P, P + N_CLASS], f32,
                                         isOutput=False)
    fc_dram = nc.declare_dram_parameter("f_convert", [C, NPIX], f32, isOutput=False)
    seg_dram = nc.declare_dram_parameter("seg", [NPIX], i32, isOutput=False)
    out_dram = nc.declare_dram_parameter("out", [2, N_CLASS, C], f32, isOutput=True)

    with tile.TileContext(nc) as tc:
        with (
            tc.tile_pool(name="const", bufs=1) as const_pool,
            tc.tile_pool(name="stage", bufs=1) as stage_pool,
            tc.tile_pool(name="work", bufs=4) as work_pool,
            tc.tile_pool(name="psum_t", bufs=5, space="PSUM") as psum_t_pool,
            tc.tile_pool(name="psum_abs", bufs=1, space="PSUM") as psum_abs_pool,
            tc.tile_pool(name="psum_acc", bufs=1, space="PSUM") as psum_acc_pool,
        ):
            # identity + iota row arrive via DMA (the "aux" input): building
            # them with gpsimd would add the Pool semaphore to the kernel-tail
            # drain's wait list, which walrus caps hard.
            aux_sb = const_pool.tile([P, P + N_CLASS], f32, tag="aux")
            nc.gpsimd.dma_start(out=aux_sb[:], in_=aux_dram[:])
            identity = aux_sb[:, 0:P]
            iota19 = aux_sb[:, P:P + N_CLASS]

            # seg laid out so partition p = pixel-within-group, free g = group.
            seg_i = const_pool.tile([P, NG], i32, tag="seg_i")
            with nc.allow_non_contiguous_dma(reason="seg is tiny (32KB)"):
                nc.gpsimd.dma_start(
                    out=seg_i[:],
                    in_=seg_dram[:].rearrange("(g p) -> p g", p=P),
                )
            seg_sb = const_pool.tile([P, NG], f32, tag="seg")
            nc.vector.tensor_copy(seg_sb[:], seg_i[:])

            # Dummy DVE read of iota19: syncs DVE against gpsimd so the first
            # w-generation op doesn't need a second (Pool) wait — the DVE
            # tensor-scalar encoding also has a single sync-wait slot.
            iota_warm = const_pool.tile([P, N_CLASS], f32, tag="iota_warm")
            nc.vector.tensor_copy(iota_warm[:], iota19)

            # Warm-up transpose: syncs PE against the gpsimd-built identity so
            # the first real transpose carries a single (DMA) wait — walrus's
            # LDWEIGHTS encoding only has one sync-wait slot.
            warm = psum_t_pool.tile([P, P], f32, tag="pt", name="warm", padded_shape=[P, 512])
            nc.tensor.transpose(warm[:, 0:P], identity, identity)

            accs = {
                "s": psum_acc_pool.tile([N_CLASS, C], f32, tag="acc_s", name="acc_s"),
                "c": psum_acc_pool.tile([N_CLASS, C], f32, tag="acc_c", name="acc_c"),
            }
            drams = {"s": fs_dram, "c": fc_dram}

            # Dedicated bank the DMA-wait absorber transposes write into
            # (never read; lo/hi slices are byte-disjoint).
            absorb = psum_abs_pool.tile([P, 4 * P], f32, tag="absorb",
                                        name="absorb")

            # PE instructions may carry only ONE sync wait (walrus S3_LW
            # limit). Strategy: (a) absorber transposes take the staging-DMA
            # waits; (b) sync=False ordering edges keep each group's
            # transposes scheduled after the matmul 5 groups back, so their
            # PSUM-slot WAR wait on DVE is subsumed by the vector clock and
            # only the (monotonic) PE WAW wait is emitted.
            mm_all = []

            def order_after_mm(inst):
                if len(mm_all) >= 5:
                    add_dep_helper(inst.ins, mm_all[-5].ins, sync=False,
                                   reason="keep PE stream near program order")

            for ci in range(NCHUNK):
                for t in ("s", "c"):
                    # One dedicated staging tile per (chunk, tensor, half):
                    # no slot reuse, so the staging DMAs carry zero waits
                    # (the 1-wait-per-instruction walrus limit again).
                    lo = stage_pool.tile([P, CHUNK_PIX], f32,
                                         tag=f"{t}_lo_{ci}", name=f"{t}_lo_{ci}")
                    hi = stage_pool.tile([P, CHUNK_PIX], f32,
                                         tag=f"{t}_hi_{ci}", name=f"{t}_hi_{ci}")
                    pix0 = ci * CHUNK_PIX
                    nc.gpsimd.dma_start(
                        out=lo[:], in_=drams[t][0:P, pix0:pix0 + CHUNK_PIX])
                    nc.gpsimd.dma_start(
                        out=hi[:], in_=drams[t][P:C, pix0:pix0 + CHUNK_PIX])
                    ab1 = nc.tensor.transpose(absorb[:, 0:P], lo[:, 0:P],
                                              identity)
                    ab2 = nc.tensor.transpose(absorb[:, P:2 * P], hi[:, 0:P],
                                              identity)
                    order_after_mm(ab1)
                    order_after_mm(ab2)
                    for g in range(GPC):
                        G = ci * GPC + g
                        psumT = psum_t_pool.tile([P, C], f32, tag="pt", padded_shape=[P, 512])
                        t1 = nc.tensor.transpose(
                            psumT[:, 0:P], lo[:, g * P:(g + 1) * P], identity)
                        t2 = nc.tensor.transpose(
                            psumT[:, P:C], hi[:, g * P:(g + 1) * P], identity)
                        order_after_mm(t1)
                        order_after_mm(t2)
                        xT = work_pool.tile([P, C], f32, tag="xT")
                        nc.vector.tensor_copy(xT[:], psumT[:])

                        sq = work_pool.tile([P, C], f32, tag="sq")
                        ss = work_pool.tile([P, 1], f32, tag="ss")
                        nc.scalar.activation(
                            sq[:], xT[:], mybir.ActivationFunctionType.Square,
                            accum_out=ss[:])
                        nrm = work_pool.tile([P, 1], f32, tag="nrm")
                        nc.scalar.sqrt(nrm[:], ss[:])
                        r = work_pool.tile([P, 1], f32, tag="r")
                        nc.vector.reciprocal(r[:], nrm[:])

                        w = work_pool.tile([P, N_CLASS], f32, tag="w")
                        nc.vector.tensor_scalar(
                            out=w[:], in0=iota19,
                            scalar1=seg_sb[:, G:G + 1], scalar2=r[:],
                            op0=mybir.AluOpType.is_equal,
                            op1=mybir.AluOpType.mult)

                        mm = nc.tensor.matmul(
                            accs[t][:], lhsT=w[:], rhs=xT[:],
                            start=(G == 0), stop=(G == NG - 1))
                        mm_all.append(mm)

            out_sb = work_pool.tile([N_CLASS, 2 * C], f32, tag="out_sb")
            nc.vector.tensor_copy(out_sb[:, 0:C], accs["s"][:])
            nc.vector.tensor_copy(out_sb[:, C:2 * C], accs["c"][:])
            # Single HWDGE DMA on an otherwise-unused lane: carries only the
            # DVE wait (the 1-sync-wait walrus limit yet again).
            nc.sync.dma_start(
                out=out_dram[:].transpose([1, 0, 2]),
                in_=out_sb[:, 0:2 * C].rearrange("b (a c) -> b a c", a=2))

    return nc


def aux_array():
    ident = np.eye(P, dtype=np.float32)
    iota = np.tile(np.arange(N_CLASS, dtype=np.float32), (P, 1))
    return np.ascontiguousarray(np.concatenate([ident, iota], axis=1))


def shard_inputs(f_source, f_convert, seg):
    """Split by (batch, h-half) into 8 per-core input maps."""
    in_maps = []
    hh = H // 2
    aux = aux_array()
    for core in range(N_CORES):
        b, half = divmod(core, 2)
        h0 = half * hh
        in_maps.append({
            "f_source": np.ascontiguousarray(
                f_source[b, :, h0:h0 + hh, :]).reshape(C, NPIX),
            "f_convert": np.ascontiguousarray(
                f_convert[b, :, h0:h0 + hh, :]).reshape(C, NPIX),
            "seg": np.ascontiguousarray(seg[b, h0:h0 + hh, :]).reshape(NPIX),
            "aux": aux,
        })
    return in_maps


def epilogue(S, Csum):
    """Tiny triplet-loss tail on [19,256] class sums (float64 host math)."""
    n = float(B * H * W)
    cs = S.astype(np.float64) / n
    cc = Csum.astype(np.float64) / n
    cs = cs / np.maximum(np.linalg.norm(cs, axis=1, keepdims=True), EPS_NORM)
    cc = cc / np.maximum(np.linalg.norm(cc, axis=1, keepdims=True), EPS_NORM)
    D = np.linalg.norm(cs[:, None, :] - cc[None, :, :] + EPS_TRIP, axis=2)
    d_ap = np.diag(D)
    terms = np.maximum(d_ap[:, None] - D + MARGIN, 0.0)
    mask = 1.0 - np.eye(N_CLASS)
    loss = (terms * mask).sum() / (N_CLASS * (N_CLASS - 1))
    return np.float32(loss)


def kernel(f_source, f_convert, seg):
    if "nc" not in _NC_CACHE:
        _NC_CACHE["nc"] = build_nc()
    nc = _NC_CACHE["nc"]
    in_maps = shard_inputs(f_source, f_convert, seg)
    res = run_bass_kernel_spmd(nc, in_maps, core_ids=list(range(N_CORES)))
    partials = np.stack([r["out"] for r in res.results])  # [8, 2, 19, 256]
    total = partials.sum(axis=0)
    return epilogue(total[0], total[1])


if __name__ == "__main__":
    rng = np.random.default_rng(0)
    fs = rng.standard_normal((B, C, H, W), dtype=np.float32)
    fc = rng.standard_normal((B, C, H, W), dtype=np.float32)
    sg = rng.integers(0, N_CLASS, size=(B, H, W), dtype=np.int32)
    print(kernel(fs, fc, sg))


# revision 22
# speedup vs baseline: 1.0115x; 1.0115x over previous
"""Class-align loss (segment_reduce) Trainium2 kernel.

Full inputs: f_source [4,256,128,128] f32, f_convert [4,256,128,128] f32,
seg [4,128,128] int32 (values in [0,19)). Output: scalar f32 triplet loss.

Strategy (data-parallel over batch*h-half, 8 shards):
  - Each core processes a [256, 8192] shard of each feature tensor
    (1 batch x 64 h-rows x 128 w).
  - Per 128-pixel group: PE-transpose the two [128c, 128p] blocks into a
    [128p, 256c] SBUF tile; ACT computes per-pixel sum-of-squares
    (Square + accum); per-pixel 1/||x|| is folded into the one-hot class
    weights w[p,k] = (seg[p]==k) * rsqrt(sumsq[p]).
  - PE accumulates S[k,c] += w.T @ xT into a [19,256] PSUM accumulator
    (64 accumulating matmuls per tensor). Same for f_convert.
  - Each core writes its partial [2,19,256] sums; the host sums the 8
    partials and computes the tiny (19-class) normalize + triplet-loss
    epilogue in float64.
"""

import sys

import numpy as np

if "/opt/trn_rl_repo" not in sys.path:
    sys.path.insert(0, "/opt/trn_rl_repo")

import concourse.bass as bass
import concourse.mybir as mybir
import concourse.tile as tile
import concourse.tile_sem_assignment as _tsa

# Cap the SWDGE semaphore round-robin: every sem a kernel touches lands in
# the kernel-tail drain's wait list, and walrus caps sync waits per
# instruction. One lane still pipelines transfers (completion counting only).
_tsa.NUM_SWDGE_GLOBAL_SEMS = 1

# This walrus build encodes at most ONE sync wait per instruction, but
# Tile's kernel-tail drain carries one wait per semaphore the kernel ever
# used. Split the excess waits onto dedicated sequencer NOPs emitted before
# the sem-clearing barrier.
from concourse.vector_clock import ScopedClock


def _split_drain_and_barrier(self, tick_clock, wait_clock):
    nc = self.nc
    drain_inst = nc.sync.drain()
    wait_clock.add_sem_waits(
        drain_inst.ins, ScopedClock({None: tick_clock.global_clock})
    )
    si = drain_inst.ins.sync_info
    if si is not None and len(si.on_wait) > 1:
        waits = list(si.on_wait)
        upds = list(si.on_update)
        drain_inst.ins.sync_info = mybir.SyncInfo(
            on_wait=waits[:1], on_update=upds)
        for k in range(1, len(waits)):
            nop = nc.sync.nop(nofuse=True, hint=f"drain_wait_{k}")
            nop.ins.sync_info = mybir.SyncInfo(
                on_wait=[waits[k]], on_update=[])
    nc.all_engine_barrier()
    assert self.sems is not None
    popped = nc._tile_sem_poison_stack.pop()
    assert popped is self._sem_poison
    nc.clear_and_free_semaphores(list(self.sems.allocated().values()))
    nc.all_engine_barrier()


tile.TileContext._drain_and_barrier = _split_drain_and_barrier
from concourse.bass_utils import run_bass_kernel_spmd
from concourse.tile import add_dep_helper

# Problem constants (hardcoded; kernel.py must be self-contained).
B, C, H, W = 4, 256, 128, 128
N_CLASS = 19
N_CORES = 8
EPS_NORM = 1e-12
EPS_TRIP = 1e-6
MARGIN = 0.2

P = 128                    # SBUF partitions / pixel-group size
NPIX = B * H * W // N_CORES  # 8192 pixels per core
NG = NPIX // P             # 64 pixel groups per core
CHUNK_PIX = 2048           # pixels staged per DMA
NCHUNK = NPIX // CHUNK_PIX # 4
GPC = CHUNK_PIX // P       # 16 groups per chunk

_NC_CACHE = {}


def build_nc():
    f32 = mybir.dt.float32
    bf16 = mybir.dt.bfloat16
    i32 = mybir.dt.int32
    nc = bass.Bass(num_swdge_queues=2)

    fs_dram = nc.declare_dram_parameter("f_source", [C, NPIX], f32, isOutput=False)
    aux_dram = nc.declare_dram_parameter("aux", [P, P + N_CLASS], f32,
                                         isOutput=False)
    fc_dram = nc.declare_dram_parameter("f_convert", [C, NPIX], f32, isOutput=False)
    seg_dram = nc.declare_dram_parameter("seg", [NPIX], i32, isOutput=False)
    out_dram = nc.declare_dram_parameter("out", [2, N_CLASS, C], f32, isOutput=True)

    with tile.TileContext(nc) as tc:
        with (
            tc.tile_pool(name="const", bufs=1) as const_pool,
            tc.tile_pool(name="stage", bufs=1) as stage_pool,
            tc.tile_pool(name="work", bufs=4) as work_pool,
            tc.tile_pool(name="psum_t", bufs=5, space="PSUM") as psum_t_pool,
            tc.tile_pool(name="psum_abs", bufs=1, space="PSUM") as psum_abs_pool,
            tc.tile_pool(name="psum_acc", bufs=1, space="PSUM") as psum_acc_pool,
        ):
            # identity + iota row arrive via DMA (the "aux" input): building
            # them with gpsimd would add the Pool semaphore to the kernel-tail
            # drain's wait list, which walrus caps hard.
            aux_sb = const_pool.tile([P, P + N_CLASS], f32, tag="aux")
            nc.gpsimd.dma_start(out=aux_sb[:], in_=aux_dram[:])
            identity_f32 = aux_sb[:, 0:P]
            iota19 = aux_sb[:, P:P + N_CLASS]
            ident_bf = const_pool.tile([P, P], bf16, tag="ident_bf")
            nc.vector.tensor_copy(ident_bf[:], identity_f32)
            identity = ident_bf[:]

            # seg laid out so partition p = pixel-within-group, free g = group.
            seg_i = const_pool.tile([P, NG], i32, tag="seg_i")
            with nc.allow_non_contiguous_dma(reason="seg is tiny (32KB)"):
                nc.gpsimd.dma_start(
                    out=seg_i[:],
                    in_=seg_dram[:].rearrange("(g p) -> p g", p=P),
                )
            seg_sb = const_pool.tile([P, NG], f32, tag="seg")
            nc.vector.tensor_copy(seg_sb[:], seg_i[:])

            # Dummy DVE read of iota19: syncs DVE against gpsimd so the first
            # w-generation op doesn't need a second (Pool) wait — the DVE
            # tensor-scalar encoding also has a single sync-wait slot.
            iota_warm = const_pool.tile([P, N_CLASS], f32, tag="iota_warm")
            nc.vector.tensor_copy(iota_warm[:], iota19)

            # Warm-up transpose: syncs PE against the gpsimd-built identity so
            # the first real transpose carries a single (DMA) wait — walrus's
            # LDWEIGHTS encoding only has one sync-wait slot.
            warm = psum_t_pool.tile([P, P], bf16, tag="pt", name="warm", padded_shape=[P, 1024])
            nc.tensor.transpose(warm[:, 0:P], identity, identity)

            accs = {
                "s": psum_acc_pool.tile([N_CLASS, C], f32, tag="acc_s", name="acc_s"),
                "c": psum_acc_pool.tile([N_CLASS, C], f32, tag="acc_c", name="acc_c"),
            }
            drams = {"s": fs_dram, "c": fc_dram}

            # Dedicated bank the DMA-wait absorber transposes write into
            # (never read; lo/hi slices are byte-disjoint).
            absorb = psum_abs_pool.tile([P, 4 * P], bf16, tag="absorb",
                                        name="absorb", padded_shape=[P, 1024])

            # PE instructions may carry only ONE sync wait (walrus S3_LW
            # limit). Strategy: (a) absorber transposes take the staging-DMA
            # waits; (b) sync=False ordering edges keep each group's
            # transposes scheduled after the matmul 5 groups back, so their
            # PSUM-slot WAR wait on DVE is subsumed by the vector clock and
            # only the (monotonic) PE WAW wait is emitted.
            mm_all = []

            def order_after_mm(inst):
                if len(mm_all) >= 5:
                    add_dep_helper(inst.ins, mm_all[-5].ins, sync=False,
                                   reason="keep PE stream near program order")

            for ci in range(NCHUNK):
                for t in ("s", "c"):
                    # One dedicated staging tile per (chunk, tensor, half):
                    # no slot reuse, so the staging DMAs carry zero waits
                    # (the 1-wait-per-instruction walrus limit again).
                    lo = stage_pool.tile([P, CHUNK_PIX], bf16,
                                         tag=f"{t}_lo_{ci}", name=f"{t}_lo_{ci}")
                    hi = stage_pool.tile([P, CHUNK_PIX], bf16,
                                         tag=f"{t}_hi_{ci}", name=f"{t}_hi_{ci}")
                    pix0 = ci * CHUNK_PIX
                    nc.gpsimd.dma_start(
                        out=lo[:], in_=drams[t][0:P, pix0:pix0 + CHUNK_PIX])
                    nc.gpsimd.dma_start(
                        out=hi[:], in_=drams[t][P:C, pix0:pix0 + CHUNK_PIX])
                    ab1 = nc.tensor.transpose(absorb[:, 0:P], lo[:, 0:P],
                                              identity)
                    ab2 = nc.tensor.transpose(absorb[:, P:2 * P], hi[:, 0:P],
                                              identity)
                    order_after_mm(ab1)
                    order_after_mm(ab2)
                    for g in range(GPC):
                        G = ci * GPC + g
                        psumT = psum_t_pool.tile([P, C], bf16, tag="pt", padded_shape=[P, 1024])
                        t1 = nc.tensor.transpose(
                            psumT[:, 0:P], lo[:, g * P:(g + 1) * P], identity)
                        t2 = nc.tensor.transpose(
                            psumT[:, P:C], hi[:, g * P:(g + 1) * P], identity)
                        order_after_mm(t1)
                        order_after_mm(t2)
                        xT = work_pool.tile([P, C], bf16, tag="xT")
                        nc.vector.tensor_copy(xT[:], psumT[:])

                        sq = work_pool.tile([P, C], bf16, tag="sq")
                        ss = work_pool.tile([P, 1], f32, tag="ss")
                        nc.scalar.activation(
                            sq[:], xT[:], mybir.ActivationFunctionType.Square,
                            accum_out=ss[:])
                        nrm = work_pool.tile([P, 1], f32, tag="nrm")
                        nc.scalar.sqrt(nrm[:], ss[:])
                        r = work_pool.tile([P, 1], f32, tag="r")
                        nc.vector.reciprocal(r[:], nrm[:])

                        w = work_pool.tile([P, N_CLASS], bf16, tag="w")
                        nc.vector.tensor_scalar(
                            out=w[:], in0=iota19,
                            scalar1=seg_sb[:, G:G + 1], scalar2=r[:],
                            op0=mybir.AluOpType.is_equal,
                            op1=mybir.AluOpType.mult)

                        mm = nc.tensor.matmul(
                            accs[t][:], lhsT=w[:], rhs=xT[:],
                            start=(G == 0), stop=(G == NG - 1))
                        mm_all.append(mm)

            out_sb = work_pool.tile([N_CLASS, 2 * C], f32, tag="out_sb")
            nc.vector.tensor_copy(out_sb[:, 0:C], accs["s"][:])
            nc.vector.tensor_copy(out_sb[:, C:2 * C], accs["c"][:])
            # Single HWDGE DMA on an otherwise-unused lane: carries only the
            # DVE wait (the 1-sync-wait walrus limit yet again).
            nc.sync.dma_start(
                out=out_dram[:].transpose([1, 0, 2]),
                in_=out_sb[:, 0:2 * C].rearrange("b (a c) -> b a c", a=2))

    return nc


def aux_array():
    ident = np.eye(P, dtype=np.float32)
    iota = np.tile(np.arange(N_CLASS, dtype=np.float32), (P, 1))
    return np.ascontiguousarray(np.concatenate([ident, iota], axis=1))


def shard_inputs(f_source, f_convert, seg):
    """Split by (batch, h-half) into 8 per-core input maps."""
    in_maps = []
    hh = H // 2
    aux = aux_array()
    for core in range(N_CORES):
        b, half = divmod(core, 2)
        h0 = half * hh
        in_maps.append({
            "f_source": np.ascontiguousarray(
                f_source[b, :, h0:h0 + hh, :]).reshape(C, NPIX),
            "f_convert": np.ascontiguousarray(
                f_convert[b, :, h0:h0 + hh, :]).reshape(C, NPIX),
            "seg": np.ascontiguousarray(seg[b, h0:h0 + hh, :]).reshape(NPIX),
            "aux": aux,
        })
    return in_maps


def epilogue(S, Csum):
    """Tiny triplet-loss tail on [19,256] class sums (float64 host math)."""
    n = float(B * H * W)
    cs = S.astype(np.float64) / n
    cc = Csum.astype(np.float64) / n
    cs = cs / np.maximum(np.linalg.norm(cs, axis=1, keepdims=True), EPS_NORM)
    cc = cc / np.maximum(np.linalg.norm(cc, axis=1, keepdims=True), EPS_NORM)
    D = np.linalg.norm(cs[:, None, :] - cc[None, :, :] + EPS_TRIP, axis=2)
    d_ap = np.diag(D)
    terms = np.maximum(d_ap[:, None] - D + MARGIN, 0.0)
    mask = 1.0 - np.eye(N_CLASS)
    loss = (terms * mask).sum() / (N_CLASS * (N_CLASS - 1))
    return np.float32(loss)


def kernel(f_source, f_convert, seg):
    if "nc" not in _NC_CACHE:
        _NC_CACHE["nc"] = build_nc()
    nc = _NC_CACHE["nc"]
    in_maps = shard_inputs(f_source, f_convert, seg)
    res = run_bass_kernel_spmd(nc, in_maps, core_ids=list(range(N_CORES)))
    partials = np.stack([r["out"] for r in res.results])  # [8, 2, 19, 256]
    total = partials.sum(axis=0)
    return epilogue(total[0], total[1])


if __name__ == "__main__":
    rng = np.random.default_rng(0)
    fs = rng.standard_normal((B, C, H, W), dtype=np.float32)
    fc = rng.standard_normal((B, C, H, W), dtype=np.float32)
    sg = rng.integers(0, N_CLASS, size=(B, H, W), dtype=np.int32)
    print(kernel(fs, fc, sg))
